# revision 29
# baseline (speedup 1.0000x reference)
"""Trainium2 Bass kernel for nn_GSCAN_model (gnn_message_passing).

Reference computation (per cell of a [B, 32, 32, 17] grid):
    emb    = concat(x[0:4] @ W_size, x[4:8] @ W_shape,
                    x[8:12] @ W_rgb, x[12:17] @ W_agent)     # [64]
    mask   = sum(x) > 0
    out    = mask ? emb : [x, zeros(47)]                     # [64]

Kernel formulation: fold the mask into the matmul.
    out = (x*m) @ (Wblk - P17)  +  pad(x)
Wblk is the 17x64 block-diagonal assembly of the four small weights;
the matmul runs in float32r (single-pass PE) and the pad/passthrough
add (+x on channels 0:17) is fused into the PSUM->SBUF drain, so no
engine sits at the tail of the per-macro chain except the drain
engines themselves.  Input DMAs issue from the ACT HWDGE ring so they
never queue behind output DMAs waiting on drains (Sync ring).

The matmul needs channels-on-partitions, so each macro tile
(128 partitions x 32 cells) runs 5 PE transposes (7 cells of 17
channels each -> 119 rows) and 5 block-diagonal matmuls against
Wd [119, 448] (7 copies of Wblk on the diagonal).

Cell-to-partition mapping is partition-outermost: partition p owns the
contiguous HBM range of cells [p*2048, (p+1)*2048), so per-partition
DMA descriptors are large and contiguous: input batches 4 macros per
DMA (8704 B/partition), output batches 2 macros (16 KB/partition).
DMA roofline: ~85 MB @ ~358 GB/s per core ~= 238 us.

Data parallel over 8 NeuronCores: batch dim 2048 -> 256 per core.
"""

import numpy as np

B, H, W, C_IN = 2048, 32, 32, 17
EMB = 64
N_CORES = 8
P = 128                      # partitions
C_SLOTS = 32                 # cells per partition per macro tile
CELLS_PER_CORE = (B // N_CORES) * H * W          # 262144
CPP = CELLS_PER_CORE // P                        # 2048 cells per partition
MACROS = CPP // C_SLOTS                          # 64
IN_BATCH = 4                 # macros per input DMA
OUT_BATCH = 2                # macros per output DMA
N_IB = MACROS // IN_BATCH    # 16
N_OB = MACROS // OUT_BATCH   # 32
# groups of cell-subtiles per macro: 4 groups of 7 slots + 1 group of 4
GROUPS = [(0, 7), (7, 7), (14, 7), (21, 7), (28, 4)]

_CACHE = {}


def _build_program():
    import concourse.bacc as bacc
    import concourse.mybir as mybir
    from concourse.tile import TileContext

    f32 = mybir.dt.float32
    f32r = mybir.dt.float32r
    nc = bacc.Bacc("TRN2", target_bir_lowering=False, debug=False,
                   num_devices=N_CORES)

    x = nc.dram_tensor("x", [CELLS_PER_CORE, C_IN], f32, kind="ExternalInput")
    wd = nc.dram_tensor("wd", [7 * C_IN, 7 * EMB], f32r,
                        kind="ExternalInput")
    ident = nc.dram_tensor("ident", [P, P], f32, kind="ExternalInput")
    y = nc.dram_tensor("y", [CELLS_PER_CORE, EMB], f32, kind="ExternalOutput")

    # partition-outermost: partition p <- cells [p*CPP, (p+1)*CPP)
    xr = x.ap().rearrange("(p b q) k -> b p (q k)", p=P, b=N_IB)
    yr = y.ap().rearrange("(p o q) n -> o p (q n)", p=P, o=N_OB)

    KMAX = 7 * C_IN              # 119 rows: largest group
    NMAX = 7 * EMB               # 448 cols

    with TileContext(nc) as tc:
        with (
            tc.tile_pool(name="const", bufs=1) as constp,
            tc.tile_pool(name="xin", bufs=6) as xin_pool,
            tc.tile_pool(name="mask", bufs=8) as mask_pool,
            tc.tile_pool(name="xm", bufs=6) as xm_pool,
            tc.tile_pool(name="xat", bufs=4) as xat_pool,
            tc.tile_pool(name="outp", bufs=7) as out_pool,
            tc.tile_pool(name="pstA", bufs=2, space="PSUM") as pstA_pool,
            tc.tile_pool(name="pstB", bufs=2, space="PSUM") as pstB_pool,
            tc.tile_pool(name="pso", bufs=4, space="PSUM") as pso_pool,
        ):
            wd_t = constp.tile([KMAX, NMAX], f32r)
            nc.sync.dma_start(out=wd_t, in_=wd.ap())
            id_t = constp.tile([P, P], f32)
            nc.sync.dma_start(out=id_t, in_=ident.ap())

            # Input prefetch: issue DMA for batch k three batches ahead of
            # its consumption so the ACT stream position never limits DMA
            # lead time (issues are in-order within an engine's stream).
            LEAD = 3
            xts = {}

            def issue_in(k):
                if k < N_IB:
                    t = xin_pool.tile([P, IN_BATCH * C_SLOTS * C_IN], f32)
                    # first few input DMAs ride the (startup-idle) Sync
                    # HWDGE ring so upfront in-flight prefetch is not
                    # capped by one ring's depth
                    eng = nc.sync if k < 5 else nc.scalar
                    eng.dma_start(out=t, in_=xr[k])
                    xts[k] = t

            for k in range(LEAD):
                issue_in(k)

            out_t = None
            for ib in range(N_IB):
                issue_in(ib + LEAD)
                xt = xts.pop(ib)
                xt4 = xt.rearrange("p (g c k) -> p g c k", g=IN_BATCH, k=C_IN)

                for g in range(IN_BATCH):
                    mi = ib * IN_BATCH + g
                    half = mi % OUT_BATCH
                    if half == 0:
                        out_t = out_pool.tile(
                            [P, OUT_BATCH * C_SLOTS * EMB], f32)
                    out4 = out_t.rearrange("p (d c e) -> p d c e",
                                           d=OUT_BATCH, e=EMB)
                    xt3 = xt4[:, g]                       # [P, 32, 17]

                    s_t = mask_pool.tile([P, C_SLOTS], f32, tag="s")
                    m_t = mask_pool.tile([P, C_SLOTS], f32, tag="m")
                    nc.vector.tensor_reduce(out=s_t, in_=xt3,
                                            axis=mybir.AxisListType.X,
                                            op=mybir.AluOpType.add)
                    nc.vector.tensor_scalar(out=m_t, in0=s_t, scalar1=0.0,
                                            scalar2=None,
                                            op0=mybir.AluOpType.is_gt)

                    # xm = x*mask (matmul operand); with Wd built from
                    # (Wblk - P17) the drain's +x on channels 0:17 yields
                    # where(m, emb, pad(x)).
                    xm = xm_pool.tile([P, C_SLOTS * C_IN], f32, tag="xm")
                    xm3 = xm.rearrange("p (c k) -> p c k", k=C_IN)
                    m_b = m_t.unsqueeze(2).broadcast_to((P, C_SLOTS, C_IN))
                    nc.gpsimd.tensor_tensor(out=xm3, in0=xt3, in1=m_b,
                                            op=mybir.AluOpType.mult)

                    # 5 PE transposes: groups 0-3 -> psum bank A, 4 -> B
                    tpA = pstA_pool.tile([P, 4 * P], f32, tag="tpA")
                    tpB = pstB_pool.tile([P, P], f32, tag="tpB")
                    for gi, (c0, ns) in enumerate(GROUPS):
                        k = ns * C_IN
                        dst = (tpA[0:k, gi * P:(gi + 1) * P] if gi < 4
                               else tpB[0:k, :])
                        nc.tensor.transpose(
                            out=dst,
                            in_=xm[:, c0 * C_IN:(c0 + ns) * C_IN],
                            identity=id_t)
                    xatA = xat_pool.tile([P, 4 * P], f32r, tag="xatA")
                    xatB = xat_pool.tile([P, P], f32r, tag="xatB")
                    nc.scalar.copy(out=xatA[0:KMAX, :], in_=tpA[0:KMAX, :])
                    nc.vector.tensor_copy(out=xatB[0:4 * C_IN, :],
                                          in_=tpB[0:4 * C_IN, :])

                    # 5 matmuls -> psum; drain fuses the passthrough add:
                    #   out[:, :, 17:64] = po  (ACT copy)
                    #   out[:, :,  0:17] = po + xinv  (DVE tensor_tensor)
                    for gi, (c0, ns) in enumerate(GROUPS):
                        k = ns * C_IN
                        n = ns * EMB
                        lhsT = (xatA[0:k, gi * P:(gi + 1) * P] if gi < 4
                                else xatB[0:k, :])
                        po = pso_pool.tile([P, NMAX], f32, tag="po")
                        nc.tensor.matmul(out=po[:, 0:n], lhsT=lhsT,
                                         rhs=wd_t[0:k, 0:n],
                                         start=True, stop=True)
                        po3 = po.rearrange("p (c e) -> p c e", e=EMB)
                        if gi == 4:
                            nc.vector.tensor_copy(
                                out=out4[:, half, c0:c0 + ns, C_IN:EMB],
                                in_=po3[:, 0:ns, C_IN:EMB])
                        else:
                            nc.scalar.copy(
                                out=out4[:, half, c0:c0 + ns, C_IN:EMB],
                                in_=po3[:, 0:ns, C_IN:EMB])
                        nc.vector.tensor_tensor(
                            out=out4[:, half, c0:c0 + ns, 0:C_IN],
                            in0=po3[:, 0:ns, 0:C_IN],
                            in1=xt3[:, c0:c0 + ns, :],
                            op=mybir.AluOpType.add)

                    if half == OUT_BATCH - 1:
                        nc.sync.dma_start(out=yr[mi // OUT_BATCH], in_=out_t)
    nc.compile()
    return nc


def _host_weights(W_size, W_shape, W_rgb, W_agent):
    """Wd [119, 448]: 7 diagonal blocks of (Wblk - P17) [17, 64].

    Per slot the kernel feeds X*m; (X*m) @ (Wblk - P17) + X equals
    where(m, emb, pad(X)) -- the +X on channels 0:17 is fused into the
    PSUM->SBUF drain.
    """
    wblk = np.zeros((C_IN, EMB), np.float32)
    wblk[0:4, 0:16] = W_size
    wblk[4:8, 16:32] = W_shape
    wblk[8:12, 32:48] = W_rgb
    wblk[12:17, 48:64] = W_agent
    pad = np.zeros((C_IN, EMB), np.float32)
    pad[np.arange(C_IN), np.arange(C_IN)] = 1.0
    w17 = wblk - pad                                 # [17, 64]
    wd = np.zeros((7 * C_IN, 7 * EMB), np.float32)
    for i in range(7):
        wd[i * C_IN:(i + 1) * C_IN, i * EMB:(i + 1) * EMB] = w17
    return wd


def kernel(situation, W_size, W_shape, W_rgb, W_agent):
    from concourse.bass_utils import run_bass_kernel_spmd

    key = "prog"
    if key not in _CACHE:
        _CACHE[key] = _build_program()
    nc = _CACHE[key]

    wd = _host_weights(np.asarray(W_size), np.asarray(W_shape),
                       np.asarray(W_rgb), np.asarray(W_agent))
    ident = np.eye(P, dtype=np.float32)

    sit = np.ascontiguousarray(np.asarray(situation), dtype=np.float32)
    bpc = B // N_CORES
    in_maps = []
    for i in range(N_CORES):
        shard = sit[i * bpc:(i + 1) * bpc].reshape(CELLS_PER_CORE, C_IN)
        in_maps.append({"x": np.ascontiguousarray(shard),
                        "wd": wd, "ident": ident})

    res = run_bass_kernel_spmd(nc, in_maps, core_ids=list(range(N_CORES)))
    out = np.empty((B, H, W, EMB), np.float32)
    for i in range(N_CORES):
        out[i * bpc:(i + 1) * bpc] = res.results[i]["y"].reshape(
            bpc, H, W, EMB)
    return out
